# revision 5
# baseline (speedup 1.0000x reference)
"""DSS (Diagonal State Space) forward pass on 8 Trainium2 NeuronCores.

Math: per channel h the DSS kernel has closed form K[j] = Re(sum_n v_n A_n^j)
with A = exp(step*Lambda) and v absorbing the complex-softmax normalization
(computed exactly on host in fp64). The causal conv y = K * u is computed
chunked: L=16384 positions -> C=32 chunks of T=512.
  - intra-chunk: lower-triangular Toeplitz matmuls with K[0..511] (+D on diag)
  - cross-chunk: chunk summaries q_n[c] = sum_m A^{-m} u[cT+m] (matmul),
    complex geometric scan state[c] = A^T * state[c-1] + q[c] (DVE),
    projection y_far[cT+t] = Re(sum_n v_n A^{t+T} state[c-1]) (matmul).
Sharding: H=512 channels split over 8 cores (64 each), B kept whole per core.
Host does fp64 coefficient prep and layout transposes; device does all
O(B*H*L*T) compute.

Scan state layout: 128 partitions = 64 state dims x 2 channel parities
(even h on partitions 0:64, odd h on 64:128); re/im in separate column blocks.
"""
import sys

sys.path.insert(0, '/opt/trn_rl_repo')

import numpy as np

import concourse.bass as bass
import concourse.mybir as mybir
from concourse.tile import TileContext
from concourse.vector_clock import ScopedClock

# ---------------------------------------------------------------- problem dims
B, H, L, N = 8, 512, 16384, 64
NCORES = 8
HC = H // NCORES          # channels per core
HP = HC // 2              # channel pairs per core
T, C = 512, 32            # chunk length / count
S = C + 1                 # scan slots (slot 0 = zero state)
EPS = 1e-7

F32 = mybir.dt.float32


# ------------------------------------------------- walrus 1-wait workarounds
def _patched_drain_and_barrier(self, tick_clock, wait_clock):
    # This walrus build rejects >1 sem wait per instruction; split the tail
    # drain's waits across SP NOPs (the global pass below catches the rest).
    nc = self.nc
    drain_inst = nc.sync.drain()
    wait_clock.add_sem_waits(
        drain_inst.ins, ScopedClock({None: tick_clock.global_clock})
    )
    si = drain_inst.ins.sync_info
    waits = list(si.on_wait or []) if si is not None else []
    if len(waits) > 1:
        si.on_wait = waits[:1]
        for w in waits[1:]:
            n = nc.sync.nop(nofuse=True, hint="drain_wait_split")
            n.ins.sync_info = mybir.SyncInfo(on_wait=[w], on_update=[])
    nc.all_engine_barrier()
    assert self.sems is not None
    popped = nc._tile_sem_poison_stack.pop()
    assert popped is self._sem_poison
    nc.clear_and_free_semaphores(list(self.sems.allocated().values()))
    nc.all_engine_barrier()


TileContext._drain_and_barrier = _patched_drain_and_barrier


def _split_multi_waits(nc, cap=1):
    """Hoist excess sem waits onto same-engine NOPs (walrus 1-wait cap)."""
    cnt = 0
    for fn in nc.m.functions:
        for bb in fn.blocks:
            out = []
            for inst in bb.instructions:
                si = inst.sync_info
                waits = list(si.on_wait) if si and si.on_wait else []
                if len(waits) > cap:
                    extra, keep = waits[:-cap], waits[-cap:]
                    for j in range(0, len(extra), cap):
                        n = mybir.InstNoOp(name=f"{inst.name}-ws{j}")
                        n.engine = inst.engine
                        n.sync_info = mybir.SyncInfo(
                            on_wait=extra[j:j + cap], on_update=[])
                        out.append(n)
                        cnt += 1
                    si.on_wait = keep
                out.append(inst)
            bb.instructions = out
    return cnt


# ---------------------------------------------------------------- host prep
def _prep_tables(W, Lambda, log_step, D):
    """fp64 closed-form DSS coefficients -> per-channel device tables.

    Returns statF (H,128,512), statKE (H,128,1536), Pre/Pim (H,N) f32.
    """
    Wc = W[..., 0].astype(np.float64) + 1j * W[..., 1].astype(np.float64)
    Lc = Lambda[..., 0].astype(np.float64) + 1j * Lambda[..., 1].astype(np.float64)
    step = np.exp(log_step.astype(np.float64))[:, None]       # (H,1)
    dl = step * Lc                                            # (H,N)
    re = dl.real
    assert (re <= 0).all(), "channels with Re(step*Lambda)>0 unsupported"
    assert (-re).max() * (T - 1) < 80, "fp32 overflow in A^{-m} tables"
    A = np.exp(dl)
    Am1 = A - 1.0
    small = np.abs(Am1) < 1e-12
    Am1s = np.where(small, 1.0, Am1)
    s = (np.exp(dl * L) - 1.0) / Am1s                         # sum_l A^l (re<=0)
    s = np.where(small, float(L), s)
    norm = np.conj(s) / (s * np.conj(s) + EPS)
    v = (Wc / Lc) * norm                                      # (H,N)

    p = np.arange(T)
    # A^{-p} for q summaries, realified: cols [Re | Im]
    Anp = np.exp(-dl[:, :, None] * p[None, None, :])          # (H,N,T)
    Ff = np.concatenate([Anp.real, Anp.imag], axis=1)         # (H,2N,T)
    # statF[h, m, tj*128+col] = Ff[h, col, tj*128+m]
    statF = np.ascontiguousarray(
        Ff.reshape(H, 2 * N, 4, 128).transpose(0, 3, 2, 1)
    ).reshape(H, 128, 512).astype(np.float32)

    # kernel table K'[j] = Re(sum_n v A^j), j in [0,T); +D at j=0
    Aj = np.exp(dl[:, :, None] * p[None, None, :])            # (H,N,T)
    Kp = np.einsum('hn,hnj->hj', v, Aj).real
    Kp[:, 0] += D.astype(np.float64)
    # Toeplitz tiles: Ktiles[h, o, m, t] = K'[o*128 + t - m] (0 if <0)
    mm = np.arange(128)
    idx = mm[None, None, :] - mm[None, :, None] + (np.arange(4) * 128)[:, None, None]
    Ktiles = np.where(idx[None] >= 0, Kp[:, np.clip(idx, 0, T - 1)], 0.0)

    # projection: VR/VI[n, p2] = Re/Im(v * A^{p2+T}), p2 in [0,T)
    Apt = np.exp(dl[:, :, None] * (p[None, None, :] + T)) * v[:, :, None]
    VR = Apt.real                                             # (H,N,T)
    VI = Apt.imag

    statKE = np.zeros((H, 128, 1536), np.float32)
    statKE[:, :, 0:512] = Ktiles.transpose(0, 2, 1, 3).reshape(H, 128, 512)
    # E stationaries at rows [par*64 : par*64+64] (par = h%2)
    statKE[0::2, 0:64, 512:1024] = VR[0::2]
    statKE[1::2, 64:128, 512:1024] = VR[1::2]
    statKE[0::2, 0:64, 1024:1536] = -VI[0::2]
    statKE[1::2, 64:128, 1024:1536] = -VI[1::2]

    P = np.exp(dl * T)                                        # (H,N)
    return statF, statKE, P.real.astype(np.float32), P.imag.astype(np.float32)


def _build_nc():
    nc = bass.Bass()
    u_d = nc.dram_tensor("u", [HC, 128, 1024], F32, kind="ExternalInput")
    sf_d = nc.dram_tensor("statF", [HC, 128, 512], F32, kind="ExternalInput")
    ske_d = nc.dram_tensor("statKE", [HC, 128, 1536], F32, kind="ExternalInput")
    pt_d = nc.dram_tensor("ptab", [128, 512], F32, kind="ExternalInput")
    y_d = nc.dram_tensor("y", [HC, 128, 1024], F32, kind="ExternalOutput")

    with TileContext(nc) as tc:
        with (
            tc.tile_pool(name="const", bufs=1) as cpool,
            tc.tile_pool(name="statf", bufs=2) as fpool,
            tc.tile_pool(name="stake", bufs=2) as kpool,
            tc.tile_pool(name="uin", bufs=3) as upool,
            tc.tile_pool(name="yout", bufs=3) as ypool,
            tc.tile_pool(name="tmp", bufs=2) as tpool,
            tc.tile_pool(name="qps", bufs=2, space="PSUM") as qpsum,
            tc.tile_pool(name="yps", bufs=4, space="PSUM") as ypsum,
        ):
            ptab = cpool.tile([128, 512], F32, tag="ptab")
            q2 = cpool.tile([128, 2 * HP * S * 8], F32, tag="q2")
            nc.sync.dma_start(out=ptab[:], in_=pt_d[:])

            # (128, comp, hp, s, b) view of the scan/state buffer
            q2v = q2[:].rearrange("p (c hp s b) -> p c hp s b", c=2, hp=HP, s=S, b=8)
            pre_t = ptab[:, 0:256].rearrange("p (hp b) -> p hp b", b=8)
            pim_t = ptab[:, 256:512].rearrange("p (hp b) -> p hp b", b=8)

            nc.vector.memset(q2v[:, 0, :, 0, :], 0.0)
            nc.vector.memset(q2v[:, 1, :, 0, :], 0.0)

            # ---- phase A: chunk summaries q for every channel
            for h in range(HC):
                hp, par = h // 2, h % 2
                pr = slice(par * 64, par * 64 + 64)
                sf = fpool.tile([128, 512], F32, tag="sf")
                nc.sync.dma_start(out=sf[:], in_=sf_d[h])
                ut = upool.tile([128, 1024], F32, tag="ut")
                nc.sync.dma_start(out=ut[:], in_=u_d[h])
                qp = qpsum.tile([128, 256], F32, tag="qp")
                for tj in range(4):
                    nc.tensor.matmul(
                        qp[:],
                        lhsT=sf[:, tj * 128:(tj + 1) * 128],
                        rhs=ut[:, tj * 256:(tj + 1) * 256],
                        start=(tj == 0), stop=(tj == 3),
                    )
                qpr = qp[0:64, :].rearrange("p (c b) -> p c b", b=8)
                qpi = qp[64:128, :].rearrange("p (c b) -> p c b", b=8)
                nc.scalar.copy(q2v[pr, 0, hp, 1:S, :], qpr)
                nc.scalar.copy(q2v[pr, 1, hp, 1:S, :], qpi)

            # ---- phase B: complex geometric scan across chunks (in-place)
            for s in range(2, S):
                prev_re = q2v[:, 0, :, s - 1, :]
                prev_im = q2v[:, 1, :, s - 1, :]
                t1 = tpool.tile([128, 256], F32, tag="t1")
                t2 = tpool.tile([128, 256], F32, tag="t2")
                t3 = tpool.tile([128, 256], F32, tag="t3")
                t4 = tpool.tile([128, 256], F32, tag="t4")
                t5 = tpool.tile([128, 256], F32, tag="t5")
                t6 = tpool.tile([128, 256], F32, tag="t6")

                def vw(t):
                    return t[:].rearrange("p (hp b) -> p hp b", b=8)

                nc.vector.tensor_mul(vw(t1), pre_t, prev_re)
                nc.vector.tensor_mul(vw(t2), pim_t, prev_im)
                nc.vector.tensor_sub(vw(t3), vw(t1), vw(t2))
                nc.vector.tensor_add(q2v[:, 0, :, s, :], vw(t3), q2v[:, 0, :, s, :])
                nc.vector.tensor_mul(vw(t4), pre_t, prev_im)
                nc.vector.tensor_mul(vw(t5), pim_t, prev_re)
                nc.vector.tensor_add(vw(t6), vw(t4), vw(t5))
                nc.vector.tensor_add(q2v[:, 1, :, s, :], vw(t6), q2v[:, 1, :, s, :])

            # ---- phase C: intra Toeplitz + far projection, evict y
            for h in range(HC):
                hp, par = h // 2, h % 2
                pr = slice(par * 64, par * 64 + 64)
                ske = kpool.tile([128, 1536], F32, tag="ske")
                nc.sync.dma_start(out=ske[:], in_=ske_d[h])
                ut = upool.tile([128, 1024], F32, tag="ut2")
                nc.sync.dma_start(out=ut[:], in_=u_d[h])
                ysb = ypool.tile([128, 1024], F32, tag="ysb")
                st_re = q2v[pr, 0, hp, 0:C, :]
                st_im = q2v[pr, 1, hp, 0:C, :]
                for g in range(2):
                    yp = ypsum.tile([128, 512], F32, tag="yp")
                    for tt in (2 * g, 2 * g + 1):
                        out = yp[:, (tt % 2) * 256:(tt % 2) * 256 + 256]
                        for tj in range(tt + 1):
                            o = tt - tj
                            nc.tensor.matmul(
                                out,
                                lhsT=ske[:, o * 128:(o + 1) * 128],
                                rhs=ut[:, tj * 256:(tj + 1) * 256],
                                start=(tj == 0), stop=False,
                            )
                        nc.tensor.matmul(
                            out,
                            lhsT=ske[pr, 512 + tt * 128:512 + (tt + 1) * 128],
                            rhs=st_re, start=False, stop=False,
                        )
                        nc.tensor.matmul(
                            out,
                            lhsT=ske[pr, 1024 + tt * 128:1024 + (tt + 1) * 128],
                            rhs=st_im, start=False, stop=True,
                        )
                    nc.scalar.copy(ysb[:, g * 512:(g + 1) * 512], yp[:])
                nc.sync.dma_start(out=y_d[h], in_=ysb[:])

    _split_multi_waits(nc)
    return nc


_NC_CACHE = None
_last_in_maps = None


def kernel(u, W, Lambda, log_step, D):
    global _NC_CACHE, _last_in_maps
    from concourse.bass_utils import run_bass_kernel_spmd

    u = np.asarray(u)
    out_dtype = u.dtype
    statF, statKE, Pre, Pim = _prep_tables(
        np.asarray(W), np.asarray(Lambda), np.asarray(log_step), np.asarray(D))

    in_maps = []
    for k in range(NCORES):
        hs = slice(k * HC, (k + 1) * HC)
        # u (B,H,L) -> (HC, m128, tj4*c32*b8)
        uk = u[:, hs, :].reshape(B, HC, C, 4, 128)
        uk = np.ascontiguousarray(uk.transpose(1, 4, 3, 2, 0)).reshape(HC, 128, 1024)
        # ptab (128, 512): rows [par*64+n], cols [hp*8+b | 256 + hp*8+b]
        pt = np.zeros((128, 512), np.float32)
        Pre_k, Pim_k = Pre[hs], Pim[hs]                       # (HC, N)
        pt[0:64, 0:256] = np.repeat(Pre_k[0::2].T, 8, axis=1)
        pt[64:128, 0:256] = np.repeat(Pre_k[1::2].T, 8, axis=1)
        pt[0:64, 256:512] = np.repeat(Pim_k[0::2].T, 8, axis=1)
        pt[64:128, 256:512] = np.repeat(Pim_k[1::2].T, 8, axis=1)
        in_maps.append({
            "u": uk.astype(np.float32),
            "statF": statF[hs],
            "statKE": statKE[hs],
            "ptab": pt,
        })

    _last_in_maps = in_maps
    if _NC_CACHE is None:
        _NC_CACHE = _build_nc()
    res = run_bass_kernel_spmd(_NC_CACHE, in_maps, core_ids=list(range(NCORES)))

    y = np.empty((B, H, L), np.float32)
    for k in range(NCORES):
        hs = slice(k * HC, (k + 1) * HC)
        yk = res.results[k]["y"].reshape(HC, 128, 4, C, B)
        y[:, hs, :] = yk.transpose(4, 0, 3, 2, 1).reshape(B, HC, L)
    return y.astype(out_dtype)
